# revision 18
# baseline (speedup 1.0000x reference)
"""BinaryAttention Trainium2 kernel: data-parallel over batch on 8 NeuronCores.

Per-core pipeline (16 batch items):
  qkvT = W^T-split-bf16 x3 matmul (q,k transposed d-major); v token-major,
  single bf16 product. sign/abs via ScalarE from PSUM; per-(b,h) scale
  c = mean|q| mean|k| /8.
  Attention computed in TRANSPOSED (key-major) layout to avoid PE transposes:
    S^T = sign(k)^T-slice @ sign(q)  (per m-tile)
    e^T = exp(c*S^T) * expBias^T     (ScalarE exp from PSUM, GPSIMD mult)
    Z   = ones^T @ e^T   (fp32r matmul, head-pair packed f=394)
    bc  = ones x (255/Z) (fp32r outer-product broadcast)
    pqT = round(e^T * bc) (DVE, RNE via +-2^23), ints 0..255 in bf16
    attnT_h = vdeq^T-contract @ pqT  (v dequant scale folded into v)
  proj folds 1/255 into weights.
"""
import numpy as np
import ml_dtypes

import concourse.bacc as bacc
import concourse.mybir as mybir
from concourse.tile import TileContext
from concourse.bass_utils import run_bass_kernel_spmd
from concourse.bass import AP
import concourse.bass as bass

N_CORES = 8
B = 128
BP = B // N_CORES          # 16 batch items per core
NT = 197                   # tokens
DIM = 768
NH = 12
HD = 64
NREL = 732
TOK = BP * NT              # 3152
F32 = mybir.dt.float32
F32R = mybir.dt.float32r
BF16 = mybir.dt.bfloat16
bf = ml_dtypes.bfloat16
EXP2_23 = 8388608.0
C0 = 1.0 / (NT * HD) / (NT * HD) / 8.0

_CACHE = {}


def _build_nc():
    nc = bacc.Bacc("TRN2", target_bir_lowering=False, debug=False, num_devices=1)
    d = {}
    d["xh"] = nc.dram_tensor("xh", [DIM, TOK], BF16, kind="ExternalInput").ap()
    d["xl"] = nc.dram_tensor("xl", [DIM, TOK], BF16, kind="ExternalInput").ap()
    d["wh"] = nc.dram_tensor("wh", [DIM, 3 * DIM], BF16, kind="ExternalInput").ap()
    d["wl"] = nc.dram_tensor("wl", [DIM, 3 * DIM], BF16, kind="ExternalInput").ap()
    d["pw"] = nc.dram_tensor("pw", [DIM, DIM], BF16, kind="ExternalInput").ap()
    d["pb"] = nc.dram_tensor("pb", [DIM], F32, kind="ExternalInput").ap()
    d["bias"] = nc.dram_tensor("bias", [NH, NT, NT], F32, kind="ExternalInput").ap()
    d["sel"] = nc.dram_tensor("sel", [128, 2], F32, kind="ExternalInput").ap()
    d["onesc"] = nc.dram_tensor("onesc", [128, 1], F32R, kind="ExternalInput").ap()
    d["onesr"] = nc.dram_tensor("onesr", [1, 128], F32R, kind="ExternalInput").ap()
    d["onesb"] = nc.dram_tensor("onesb", [1, 128], BF16, kind="ExternalInput").ap()
    d["pbb"] = nc.dram_tensor("pbb", [1, DIM], BF16, kind="ExternalInput").ap()
    d["out"] = nc.dram_tensor("out", [TOK, DIM], F32, kind="ExternalOutput").ap()
    cscr = nc.dram_tensor("cscr", [BP, 12], F32)

    ntl = [128, 69]   # token/m tile sizes
    noff = [0, 128]
    LAG = 4

    with TileContext(nc) as tc:
        with (
            tc.tile_pool(name="singles", bufs=1) as singles,
            tc.tile_pool(name="xpool", bufs=2) as xpool,
            tc.tile_pool(name="bpool", bufs=2) as bpool,
            tc.tile_pool(name="hpool", bufs=5) as hpool,
            tc.tile_pool(name="psA", bufs=2, space="PSUM") as psA,
            tc.tile_pool(name="psSB", bufs=3, space="PSUM") as psSB,
            tc.tile_pool(name="psZ", bufs=1, space="PSUM") as psZ,
            tc.tile_pool(name="psP", bufs=2, space="PSUM") as psP,
        ):
            # ---- resident weights/constants ----
            whs = singles.tile([128, 6, 3 * DIM], BF16, tag="whs")
            wls = singles.tile([128, 6, 3 * DIM], BF16, tag="wls")
            nc.sync.dma_start(out=whs[:], in_=d["wh"].rearrange("(k p) n -> p k n", p=128))
            nc.sync.dma_start(out=wls[:], in_=d["wl"].rearrange("(k p) n -> p k n", p=128))
            pws = singles.tile([128, 6, DIM], BF16, tag="pws")
            nc.sync.dma_start(out=pws[:], in_=d["pw"].rearrange("(k p) n -> p k n", p=128))
            # exp(bias)^T tiles: ebT[mt][m-part, h, n]
            ebT0 = singles.tile([128, NH, NT], F32, tag="ebT0")
            ebT1 = singles.tile([128, NH, NT], F32, tag="ebT1")
            nc.sync.dma_start(out=ebT0[:], in_=d["bias"][:, 0:128, :].rearrange("h m n -> m h n"))
            nc.sync.dma_start(out=ebT1[:69], in_=d["bias"][:, 128:NT, :].rearrange("h m n -> m h n"))
            ebT = [ebT0, ebT1]
            onesb = singles.tile([1, 128], BF16, tag="onesb")
            nc.sync.dma_start(out=onesb[:], in_=d["onesb"])
            pbb = singles.tile([1, DIM], BF16, tag="pbb")
            nc.sync.dma_start(out=pbb[:], in_=d["pbb"])
            sels = singles.tile([128, 2], F32, tag="sels")
            nc.sync.dma_start(out=sels[:], in_=d["sel"])
            onesc = singles.tile([128, 1], F32R, tag="onesc")
            nc.sync.dma_start(out=onesc[:], in_=d["onesc"])
            onesr = singles.tile([1, 128], F32R, tag="onesr")
            nc.sync.dma_start(out=onesr[:], in_=d["onesr"])

            pair_tiles = {}

            def make_pair_tiles(bb):
                c2 = 2 * NT
                xh_t = xpool.tile([128, 6, c2], BF16, tag="xh", name=f"xh_{bb}")
                xl_t = xpool.tile([128, 6, c2], BF16, tag="xl", name=f"xl_{bb}")
                nc.sync.dma_start(out=xh_t[:], in_=d["xh"].rearrange("(k p) t -> p k t", p=128)[:, :, bb * c2:(bb + 1) * c2])
                nc.sync.dma_start(out=xl_t[:], in_=d["xl"].rearrange("(k p) t -> p k t", p=128)[:, :, bb * c2:(bb + 1) * c2])
                sgn = [bpool.tile([128, NH, NT], BF16, tag=f"sgn{i}", name=f"sgn{i}_{bb}") for i in range(2)]
                absc = [bpool.tile([128, 12], F32, tag=f"absc{i}", name=f"absc{i}_{bb}") for i in range(2)]
                dump = [bpool.tile([128, NT], F32, tag=f"dump{i}", name=f"dump{i}_{bb}") for i in range(2)]
                pair_tiles[bb] = (xh_t, xl_t, sgn, absc, dump)

            def emit_jtile(bb, j):
                xh_t, xl_t, sgn, absc, dump = pair_tiles[bb]
                c2 = 2 * NT
                pa = psA.tile([128, c2], F32, tag="A")
                for k in range(6):
                    wj = slice(j * 128, (j + 1) * 128)
                    first = (k == 0)
                    nc.tensor.matmul(pa[:], whs[:, k, wj], xh_t[:, k, :], start=first, stop=False)
                    nc.tensor.matmul(pa[:], whs[:, k, wj], xl_t[:, k, :], start=False, stop=False)
                    nc.tensor.matmul(pa[:], wls[:, k, wj], xh_t[:, k, :], start=False, stop=(k == 5))
                for i in range(2):
                    sl = slice(i * NT, (i + 1) * NT)
                    nc.scalar.activation(out=sgn[i][:, j, :], in_=pa[:, sl], func=mybir.ActivationFunctionType.Sign)
                    nc.scalar.activation(out=dump[i][:], in_=pa[:, sl], func=mybir.ActivationFunctionType.Abs,
                                         accum_out=absc[i][:, j:j + 1])

            make_pair_tiles(0)
            for j in range(12):
                emit_jtile(0, j)

            for bb in range(BP // 2):   # pairs of batch items
                c2 = 2 * NT
                xh_t, xl_t, sgn, absc, dump = pair_tiles.pop(bb)
                if bb + 1 < BP // 2:
                    make_pair_tiles(bb + 1)

                # ---- per-item v + c stats (both items before attention) ----
                vd_i = {}
                cbc_i = {}
                rs_i = {}
                for i in range(2):
                    b = bb * 2 + i
                    vd = [bpool.tile([128, DIM], BF16, tag=f"vd{t}", name=f"vd{t}_{i}") for t in range(2)]
                    vd_i[i] = vd
                    rsA = [bpool.tile([128, 12], F32, tag=f"rsA{t}", name=f"rsA{t}_{i}") for t in range(2)]
                    rs_i[i] = rsA
                    for t in range(2):
                        tn = ntl[t]
                        xoff = i * NT + noff[t]
                        for ch in range(2):
                            vj = slice(1536 + ch * 384, 1536 + (ch + 1) * 384)
                            hs = slice(ch * 6, (ch + 1) * 6)
                            pv = psA.tile([128, 384], F32, tag="A")
                            for k in range(6):
                                nc.tensor.matmul(pv[:tn], xh_t[:, k, xoff:xoff + tn], whs[:, k, vj],
                                                 start=(k == 0), stop=(k == 5))
                            vq32 = bpool.tile([128, 384], F32, tag="vq32")
                            vmax = bpool.tile([128, 6], F32, tag="vmax")
                            ss = bpool.tile([128, 6], F32, tag="ss")
                            nc.vector.tensor_scalar(out=vq32[:tn], in0=pv[:tn], scalar1=2.0, scalar2=-2.0,
                                                    op0=mybir.AluOpType.min, op1=mybir.AluOpType.max)
                            nc.vector.tensor_reduce(out=vmax[:tn], in_=vq32[:tn].rearrange("p (h d) -> p h d", h=6),
                                                    axis=mybir.AxisListType.X, op=mybir.AluOpType.max,
                                                    apply_absolute_value=True)
                            nc.vector.tensor_scalar(out=rsA[t][:tn, hs], in0=vmax[:tn], scalar1=1e-8, scalar2=1.0 / 127.0,
                                                    op0=mybir.AluOpType.add, op1=mybir.AluOpType.mult)
                            nc.vector.reciprocal(out=ss[:tn], in_=rsA[t][:tn, hs])

                            def hbc(base_ap):
                                return AP(tensor=base_ap.tensor, offset=base_ap.offset,
                                          ap=[[int(s_), int(c_)] for s_, c_ in base_ap.ap] + [[0, HD]])
                            v3 = vq32[:tn].rearrange("p (h d) -> p h d", h=6)
                            nc.vector.tensor_tensor(out=v3, in0=v3, in1=hbc(ss[:tn]), op=mybir.AluOpType.mult)
                            nc.vector.tensor_scalar(out=vq32[:tn], in0=vq32[:tn], scalar1=EXP2_23, scalar2=EXP2_23,
                                                    op0=mybir.AluOpType.add, op1=mybir.AluOpType.subtract)
                            vdv = vd[t][:tn, ch * 384:(ch + 1) * 384].rearrange("p (h d) -> p h d", h=6)
                            nc.vector.tensor_tensor(out=vdv, in0=v3, in1=hbc(rsA[t][:tn, hs]), op=mybir.AluOpType.mult)
                    cst = psZ.tile([2, 12], F32, tag="Z")
                    nc.tensor.matmul(cst[:], sels[:], absc[i][:], start=True, stop=True)
                    css = bpool.tile([2, 12], F32, tag="css")
                    nc.vector.tensor_copy(css[:], cst[:])
                    csb = bpool.tile([2, 6], F32, tag="csb")
                    nc.vector.tensor_tensor(out=csb[:], in0=css[:2, 0:6], in1=css[:2, 6:12], op=mybir.AluOpType.mult)
                    nc.vector.tensor_scalar_mul(csb[:], csb[:], C0)
                    nc.sync.dma_start(out=cscr.ap()[b].rearrange("(r j) -> r j", r=2), in_=csb[:])
                    cbc = bpool.tile([128, 12], F32, tag="cbc", name=f"cbc{i}")
                    nc.gpsimd.dma_start(out=cbc[:], in_=AP(tensor=cscr, offset=b * 12, ap=[[0, 128], [1, 12]]))
                    cbc_i[i] = cbc

                # ---- attention: both items interleaved; step s -> (i=s%2, hp=s//2) ----
                attnT_i = {0: bpool.tile([128, 6, NT], BF16, tag="attnT", name="attnT0"),
                           1: bpool.tile([128, 6, NT], BF16, tag="attnT", name="attnT1")}
                eTs = {}
                rzs = {}

                def emit_proj(i):
                    tb = (bb * 2 + i) * NT
                    attnT = attnT_i[i]
                    osb = [bpool.tile([128, DIM], F32, tag=f"osb{t}", name=f"osb{t}_{i}") for t in range(2)]
                    for t in range(2):
                        tn = ntl[t]
                        for ch in range(2):
                            pp = psP.tile([128, 384], F32, tag="P")
                            for jt in range(6):
                                nc.tensor.matmul(pp[:tn], attnT[:, jt, noff[t]:noff[t] + tn],
                                                 pws[:, jt, ch * 384:(ch + 1) * 384], start=(jt == 0), stop=False)
                            nc.tensor.matmul(pp[:tn], onesb[:1, :tn], pbb[:1, ch * 384:(ch + 1) * 384],
                                             start=False, stop=True)
                            nc.scalar.copy(osb[t][:tn, ch * 384:(ch + 1) * 384], pp[:tn])
                        nc.sync.dma_start(out=d["out"][tb + noff[t]:tb + noff[t] + tn, :], in_=osb[t][:tn])

                for s in range(12 + LAG):
                    if s < 12 and bb + 1 < BP // 2:
                        emit_jtile(bb + 1, s)
                    if s >= LAG:
                        so = s - LAG
                        io, hpo = so % 2, so // 2
                        eA = eTs[so]
                        pz = psZ.tile([1, 2 * NT], F32, tag="Z")
                        for mt in range(2):
                            mc = ntl[mt]
                            nc.tensor.matmul(pz[:1], onesc[:mc], eA[mt][:mc],
                                             start=(mt == 0), stop=(mt == 1))
                        rzf = hpool.tile([1, 2 * NT], F32, tag="rzf")
                        nc.vector.reciprocal(out=rzf[:1], in_=pz[:1])
                        rzv = hpool.tile([1, 2 * NT], F32R, tag="rzv")
                        nc.vector.tensor_scalar_mul(rzv[:1], rzf[:1], 255.0)
                        rzs[so] = rzv
                    if s < 12:
                        i_n, hp_n = s % 2, s // 2
                        sgn_n = sgn[i_n]
                        cbc_n = cbc_i[i_n]
                        eA_n = [hpool.tile([128, 2, NT], F32R, tag=f"eA{mt}", name=f"eA{mt}_{s}")
                                for mt in range(2)]
                        for mt in range(2):
                            mc = ntl[mt]
                            mo = noff[mt]
                            ps = psSB.tile([128, 2, NT], F32, tag="SB")
                            for hh in range(2):
                                base = hh * 64
                                nc.tensor.matmul(ps[:mc, hh, :], sgn_n[base:base + 64, 6 + hp_n, mo:mo + mc],
                                                 sgn_n[base:base + 64, hp_n, :], start=True, stop=True)
                                cidx = hh * 6 + hp_n
                                nc.scalar.activation(out=eA_n[mt][:mc, hh, :], in_=ps[:mc, hh, :],
                                                     func=mybir.ActivationFunctionType.Exp,
                                                     scale=cbc_n[:mc, cidx:cidx + 1])
                            nc.gpsimd.tensor_tensor(out=eA_n[mt][:mc], in0=eA_n[mt][:mc],
                                                    in1=ebT[mt][:mc, 2 * hp_n:2 * hp_n + 2, :],
                                                    op=mybir.AluOpType.mult)
                        eTs[s] = eA_n
                    if s >= LAG:
                        so = s - LAG
                        io, hpo = so % 2, so // 2
                        eA = eTs.pop(so)
                        rzv = rzs.pop(so)
                        vd = vd_i[io]
                        bc = psSB.tile([128, 2 * NT], F32, tag="SB")
                        nc.tensor.matmul(bc[:], onesr[:1], rzv[:1], start=True, stop=True)
                        pqT = [hpool.tile([128, 2, NT], BF16, tag=f"pqT{mt}", name=f"pqT{mt}_{so}")
                               for mt in range(2)]
                        for mt in range(2):
                            mc = ntl[mt]
                            bcv = bc[:mc].rearrange("p (h n) -> p h n", h=2)
                            nc.vector.tensor_tensor(out=eA[mt][:mc], in0=eA[mt][:mc],
                                                    in1=bcv, op=mybir.AluOpType.mult)
                            nc.vector.tensor_scalar(out=pqT[mt][:mc], in0=eA[mt][:mc],
                                                    scalar1=EXP2_23, scalar2=EXP2_23,
                                                    op0=mybir.AluOpType.add, op1=mybir.AluOpType.subtract)
                        ppv = psP.tile([128, NT], F32, tag="P")
                        for hh in range(2):
                            h = 2 * hpo + hh
                            base = hh * 64
                            for mt in range(2):
                                mc = ntl[mt]
                                nc.tensor.matmul(ppv[base:base + 64, :], vd[mt][:mc, h * 64:(h + 1) * 64],
                                                 pqT[mt][:mc, hh, :], start=(mt == 0), stop=(mt == 1))
                        nc.scalar.copy(attnT_i[io][:, hpo, :], ppv[:])
                        if so == 10:
                            emit_proj(0)
                        elif so == 11:
                            emit_proj(1)
    nc.compile()
    return nc


def _build_rel_index():
    H_IN = W_IN = 14
    coords = np.stack(np.meshgrid(np.arange(H_IN), np.arange(W_IN), indexing="ij"))
    flat = coords.reshape(2, -1)
    rel = flat[:, :, None] - flat[:, None, :]
    rel = rel.transpose(1, 2, 0).astype(np.int64)
    rel[:, :, 0] += H_IN - 1
    rel[:, :, 1] += W_IN - 1
    rel[:, :, 0] *= 2 * W_IN - 1
    idx = np.zeros((NT, NT), dtype=np.int64)
    idx[1:, 1:] = rel.sum(-1)
    idx[0, :] = NREL - 3
    idx[:, 0] = NREL - 2
    idx[0, 0] = NREL - 1
    return idx


def kernel(x, qkv_w, proj_w, proj_b, rel_bias_table, rel_index):
    x = np.asarray(x, dtype=np.float32)
    qkv_w = np.asarray(qkv_w, dtype=np.float32)
    proj_w = np.asarray(proj_w, dtype=np.float32)
    proj_b = np.asarray(proj_b, dtype=np.float32)
    rel_bias_table = np.asarray(rel_bias_table, dtype=np.float32)
    rel_index = np.asarray(rel_index)

    if "nc" not in _CACHE:
        _CACHE["nc"] = _build_nc()
    nc = _CACHE["nc"]

    W2 = np.ascontiguousarray(qkv_w.T)                      # (768, 2304)
    wh = W2.astype(bf)
    wl = (W2 - wh.astype(np.float32)).astype(bf)
    pw = np.ascontiguousarray(proj_w.T / 255.0).astype(bf)  # fold 1/255
    biasg = rel_bias_table[rel_index].transpose(2, 0, 1).astype(np.float32)  # (12,197,197) [h,n,m]
    ebias = np.ascontiguousarray(np.exp(biasg.transpose(0, 2, 1)))           # (12,197,197) [h,m,n]
    sel = np.zeros((128, 2), np.float32)
    sel[:64, 0] = 1.0
    sel[64:, 1] = 1.0

    in_maps = []
    for c in range(N_CORES):
        xc = x[c * BP:(c + 1) * BP].reshape(TOK, DIM)
        xT = np.ascontiguousarray(xc.T)                     # (768, 3152)
        xh = xT.astype(bf)
        xl = (xT - xh.astype(np.float32)).astype(bf)
        in_maps.append({
            "xh": xh, "xl": xl, "wh": wh, "wl": wl, "pw": pw,
            "pb": proj_b.astype(np.float32), "bias": ebias,
            "sel": sel, "onesc": np.ones((128, 1), np.float32),
            "onesr": np.ones((1, 128), np.float32),
            "onesb": np.ones((1, 128), dtype=bf),
            "pbb": proj_b.reshape(1, DIM).astype(bf),
        })

    global _LAST_IN_MAPS
    _LAST_IN_MAPS = in_maps
    res = run_bass_kernel_spmd(nc, in_maps, list(range(N_CORES)))
    out = np.concatenate(
        [res.results[c]["out"].reshape(BP, NT, DIM) for c in range(N_CORES)], axis=0)
    return out.astype(np.float32)


# revision 19
# speedup vs baseline: 1.0664x; 1.0664x over previous
"""BinaryAttention Trainium2 kernel: data-parallel over batch on 8 NeuronCores.

Per-core pipeline (16 batch items):
  qkvT = W^T-split-bf16 x3 matmul (q,k transposed d-major); v token-major,
  single bf16 product. sign/abs via ScalarE from PSUM; per-(b,h) scale
  c = mean|q| mean|k| /8.
  Attention computed in TRANSPOSED (key-major) layout to avoid PE transposes:
    S^T = sign(k)^T-slice @ sign(q)  (per m-tile)
    e^T = exp(c*S^T) * expBias^T     (ScalarE exp from PSUM, GPSIMD mult)
    Z   = ones^T @ e^T   (fp32r matmul, head-pair packed f=394)
    bc  = ones x (255/Z) (fp32r outer-product broadcast)
    pqT = round(e^T * bc) (DVE, RNE via +-2^23), ints 0..255 in bf16
    attnT_h = vdeq^T-contract @ pqT  (v dequant scale folded into v)
  proj folds 1/255 into weights.
"""
import numpy as np
import ml_dtypes

import concourse.bacc as bacc
import concourse.mybir as mybir
from concourse.tile import TileContext
from concourse.bass_utils import run_bass_kernel_spmd
from concourse.bass import AP
import concourse.bass as bass

N_CORES = 8
B = 128
BP = B // N_CORES          # 16 batch items per core
NT = 197                   # tokens
DIM = 768
NH = 12
HD = 64
NREL = 732
TOK = BP * NT              # 3152
F32 = mybir.dt.float32
F32R = mybir.dt.float32r
BF16 = mybir.dt.bfloat16
bf = ml_dtypes.bfloat16
EXP2_23 = 8388608.0
C0 = 1.0 / (NT * HD) / (NT * HD) / 8.0

_CACHE = {}


def _build_nc():
    nc = bacc.Bacc("TRN2", target_bir_lowering=False, debug=False, num_devices=1)
    d = {}
    d["xh"] = nc.dram_tensor("xh", [DIM, TOK], BF16, kind="ExternalInput").ap()
    d["xl"] = nc.dram_tensor("xl", [DIM, TOK], BF16, kind="ExternalInput").ap()
    d["wh"] = nc.dram_tensor("wh", [DIM, 3 * DIM], BF16, kind="ExternalInput").ap()
    d["wl"] = nc.dram_tensor("wl", [DIM, 3 * DIM], BF16, kind="ExternalInput").ap()
    d["pw"] = nc.dram_tensor("pw", [DIM, DIM], BF16, kind="ExternalInput").ap()
    d["pb"] = nc.dram_tensor("pb", [DIM], F32, kind="ExternalInput").ap()
    d["bias"] = nc.dram_tensor("bias", [NH, NT, NT], F32, kind="ExternalInput").ap()
    d["sel"] = nc.dram_tensor("sel", [128, 2], F32, kind="ExternalInput").ap()
    d["onesc"] = nc.dram_tensor("onesc", [128, 1], F32R, kind="ExternalInput").ap()
    d["onesr"] = nc.dram_tensor("onesr", [1, 128], F32R, kind="ExternalInput").ap()
    d["onesb"] = nc.dram_tensor("onesb", [1, 128], BF16, kind="ExternalInput").ap()
    d["pbb"] = nc.dram_tensor("pbb", [1, DIM], BF16, kind="ExternalInput").ap()
    d["out"] = nc.dram_tensor("out", [TOK, DIM], F32, kind="ExternalOutput").ap()
    cscr = nc.dram_tensor("cscr", [BP, 12], F32)

    ntl = [128, 69]   # token/m tile sizes
    noff = [0, 128]
    LAG = 4

    with TileContext(nc) as tc:
        with (
            tc.tile_pool(name="singles", bufs=1) as singles,
            tc.tile_pool(name="xpool", bufs=2) as xpool,
            tc.tile_pool(name="bpool", bufs=2) as bpool,
            tc.tile_pool(name="hpool", bufs=5) as hpool,
            tc.tile_pool(name="psA", bufs=2, space="PSUM") as psA,
            tc.tile_pool(name="psSB", bufs=3, space="PSUM") as psSB,
            tc.tile_pool(name="psZ", bufs=1, space="PSUM") as psZ,
            tc.tile_pool(name="psP", bufs=2, space="PSUM") as psP,
        ):
            # ---- resident weights/constants ----
            whs = singles.tile([128, 6, 3 * DIM], BF16, tag="whs")
            wls = singles.tile([128, 6, 3 * DIM], BF16, tag="wls")
            nc.sync.dma_start(out=whs[:], in_=d["wh"].rearrange("(k p) n -> p k n", p=128))
            nc.sync.dma_start(out=wls[:], in_=d["wl"].rearrange("(k p) n -> p k n", p=128))
            pws = singles.tile([128, 6, DIM], BF16, tag="pws")
            nc.sync.dma_start(out=pws[:], in_=d["pw"].rearrange("(k p) n -> p k n", p=128))
            # exp(bias)^T tiles: ebT[mt][m-part, h, n]
            ebT0 = singles.tile([128, NH, NT], F32, tag="ebT0")
            ebT1 = singles.tile([128, NH, NT], F32, tag="ebT1")
            nc.sync.dma_start(out=ebT0[:], in_=d["bias"][:, 0:128, :].rearrange("h m n -> m h n"))
            nc.sync.dma_start(out=ebT1[:69], in_=d["bias"][:, 128:NT, :].rearrange("h m n -> m h n"))
            ebT = [ebT0, ebT1]
            onesb = singles.tile([1, 128], BF16, tag="onesb")
            nc.sync.dma_start(out=onesb[:], in_=d["onesb"])
            pbb = singles.tile([1, DIM], BF16, tag="pbb")
            nc.sync.dma_start(out=pbb[:], in_=d["pbb"])
            sels = singles.tile([128, 2], F32, tag="sels")
            nc.sync.dma_start(out=sels[:], in_=d["sel"])
            onesc = singles.tile([128, 1], F32R, tag="onesc")
            nc.sync.dma_start(out=onesc[:], in_=d["onesc"])
            onesr = singles.tile([1, 128], F32R, tag="onesr")
            nc.sync.dma_start(out=onesr[:], in_=d["onesr"])

            for bb in range(BP // 2):   # pairs of batch items
                c2 = 2 * NT
                xh_t = xpool.tile([128, 6, c2], BF16, tag="xh")
                xl_t = xpool.tile([128, 6, c2], BF16, tag="xl")
                nc.sync.dma_start(out=xh_t[:], in_=d["xh"].rearrange("(k p) t -> p k t", p=128)[:, :, bb * c2:(bb + 1) * c2])
                nc.sync.dma_start(out=xl_t[:], in_=d["xl"].rearrange("(k p) t -> p k t", p=128)[:, :, bb * c2:(bb + 1) * c2])

                sgn = [bpool.tile([128, NH, NT], BF16, tag=f"sgn{i}", name=f"sgn{i}") for i in range(2)]
                absc = [bpool.tile([128, 12], F32, tag=f"absc{i}", name=f"absc{i}") for i in range(2)]
                dump = [bpool.tile([128, NT], F32, tag=f"dump{i}", name=f"dump{i}") for i in range(2)]

                # ---- stage A: q,k transposed (12 j-tiles of 128 rows) ----
                for j in range(12):
                    pa = psA.tile([128, c2], F32, tag="A")
                    for k in range(6):
                        wj = slice(j * 128, (j + 1) * 128)
                        first = (k == 0)
                        nc.tensor.matmul(pa[:], whs[:, k, wj], xh_t[:, k, :], start=first, stop=False)
                        nc.tensor.matmul(pa[:], whs[:, k, wj], xl_t[:, k, :], start=False, stop=False)
                        nc.tensor.matmul(pa[:], wls[:, k, wj], xh_t[:, k, :], start=False, stop=(k == 5))
                    for i in range(2):
                        sl = slice(i * NT, (i + 1) * NT)
                        nc.scalar.activation(out=sgn[i][:, j, :], in_=pa[:, sl], func=mybir.ActivationFunctionType.Sign)
                        nc.scalar.activation(out=dump[i][:], in_=pa[:, sl], func=mybir.ActivationFunctionType.Abs,
                                             accum_out=absc[i][:, j:j + 1])

                # ---- per-item v + c stats (both items before attention) ----
                vd_i = {}
                cbc_i = {}
                rs_i = {}
                for i in range(2):
                    b = bb * 2 + i
                    vd = [bpool.tile([128, DIM], BF16, tag=f"vd{t}", name=f"vd{t}_{i}") for t in range(2)]
                    vd_i[i] = vd
                    rsA = [bpool.tile([128, 12], F32, tag=f"rsA{t}", name=f"rsA{t}_{i}") for t in range(2)]
                    rs_i[i] = rsA
                    for t in range(2):
                        tn = ntl[t]
                        xoff = i * NT + noff[t]
                        for ch in range(2):
                            vj = slice(1536 + ch * 384, 1536 + (ch + 1) * 384)
                            hs = slice(ch * 6, (ch + 1) * 6)
                            pv = psA.tile([128, 384], F32, tag="A")
                            for k in range(6):
                                nc.tensor.matmul(pv[:tn], xh_t[:, k, xoff:xoff + tn], whs[:, k, vj],
                                                 start=(k == 0), stop=(k == 5))
                            vq32 = bpool.tile([128, 384], F32, tag="vq32")
                            vmax = bpool.tile([128, 6], F32, tag="vmax")
                            ss = bpool.tile([128, 6], F32, tag="ss")
                            nc.vector.tensor_scalar(out=vq32[:tn], in0=pv[:tn], scalar1=2.0, scalar2=-2.0,
                                                    op0=mybir.AluOpType.min, op1=mybir.AluOpType.max)
                            nc.vector.tensor_reduce(out=vmax[:tn], in_=vq32[:tn].rearrange("p (h d) -> p h d", h=6),
                                                    axis=mybir.AxisListType.X, op=mybir.AluOpType.max,
                                                    apply_absolute_value=True)
                            nc.vector.tensor_scalar(out=rsA[t][:tn, hs], in0=vmax[:tn], scalar1=1e-8, scalar2=1.0 / 127.0,
                                                    op0=mybir.AluOpType.add, op1=mybir.AluOpType.mult)
                            nc.vector.reciprocal(out=ss[:tn], in_=rsA[t][:tn, hs])

                            def hbc(base_ap):
                                return AP(tensor=base_ap.tensor, offset=base_ap.offset,
                                          ap=[[int(s_), int(c_)] for s_, c_ in base_ap.ap] + [[0, HD]])
                            v3 = vq32[:tn].rearrange("p (h d) -> p h d", h=6)
                            nc.vector.tensor_tensor(out=v3, in0=v3, in1=hbc(ss[:tn]), op=mybir.AluOpType.mult)
                            nc.vector.tensor_scalar(out=vq32[:tn], in0=vq32[:tn], scalar1=EXP2_23, scalar2=EXP2_23,
                                                    op0=mybir.AluOpType.add, op1=mybir.AluOpType.subtract)
                            vdv = vd[t][:tn, ch * 384:(ch + 1) * 384].rearrange("p (h d) -> p h d", h=6)
                            nc.vector.tensor_tensor(out=vdv, in0=v3, in1=hbc(rsA[t][:tn, hs]), op=mybir.AluOpType.mult)
                    cst = psZ.tile([2, 12], F32, tag="Z")
                    nc.tensor.matmul(cst[:], sels[:], absc[i][:], start=True, stop=True)
                    css = bpool.tile([2, 12], F32, tag="css")
                    nc.vector.tensor_copy(css[:], cst[:])
                    csb = bpool.tile([2, 6], F32, tag="csb")
                    nc.vector.tensor_tensor(out=csb[:], in0=css[:2, 0:6], in1=css[:2, 6:12], op=mybir.AluOpType.mult)
                    nc.vector.tensor_scalar_mul(csb[:], csb[:], C0)
                    nc.sync.dma_start(out=cscr.ap()[b].rearrange("(r j) -> r j", r=2), in_=csb[:])
                    cbc = bpool.tile([128, 12], F32, tag="cbc", name=f"cbc{i}")
                    nc.gpsimd.dma_start(out=cbc[:], in_=AP(tensor=cscr, offset=b * 12, ap=[[0, 128], [1, 12]]))
                    cbc_i[i] = cbc

                # ---- attention: both items interleaved; step s -> (i=s%2, hp=s//2) ----
                attnT_i = {0: bpool.tile([128, 6, NT], BF16, tag="attnT", name="attnT0"),
                           1: bpool.tile([128, 6, NT], BF16, tag="attnT", name="attnT1")}
                eTs = {}
                rzs = {}

                def emit_proj(i):
                    tb = (bb * 2 + i) * NT
                    attnT = attnT_i[i]
                    osb = [bpool.tile([128, DIM], F32, tag=f"osb{t}", name=f"osb{t}_{i}") for t in range(2)]
                    for t in range(2):
                        tn = ntl[t]
                        for ch in range(2):
                            pp = psP.tile([128, 384], F32, tag="P")
                            for jt in range(6):
                                nc.tensor.matmul(pp[:tn], attnT[:, jt, noff[t]:noff[t] + tn],
                                                 pws[:, jt, ch * 384:(ch + 1) * 384], start=(jt == 0), stop=False)
                            nc.tensor.matmul(pp[:tn], onesb[:1, :tn], pbb[:1, ch * 384:(ch + 1) * 384],
                                             start=False, stop=True)
                            nc.scalar.copy(osb[t][:tn, ch * 384:(ch + 1) * 384], pp[:tn])
                        nc.sync.dma_start(out=d["out"][tb + noff[t]:tb + noff[t] + tn, :], in_=osb[t][:tn])

                for s in range(12 + LAG):
                    if s >= LAG:
                        so = s - LAG
                        io, hpo = so % 2, so // 2
                        eA = eTs[so]
                        pz = psZ.tile([1, 2 * NT], F32, tag="Z")
                        for mt in range(2):
                            mc = ntl[mt]
                            nc.tensor.matmul(pz[:1], onesc[:mc], eA[mt][:mc],
                                             start=(mt == 0), stop=(mt == 1))
                        rzf = hpool.tile([1, 2 * NT], F32, tag="rzf")
                        nc.vector.reciprocal(out=rzf[:1], in_=pz[:1])
                        rzv = hpool.tile([1, 2 * NT], F32R, tag="rzv")
                        nc.vector.tensor_scalar_mul(rzv[:1], rzf[:1], 255.0)
                        rzs[so] = rzv
                    if s < 12:
                        i_n, hp_n = s % 2, s // 2
                        sgn_n = sgn[i_n]
                        cbc_n = cbc_i[i_n]
                        eA_n = [hpool.tile([128, 2, NT], F32R, tag=f"eA{mt}", name=f"eA{mt}_{s}")
                                for mt in range(2)]
                        for mt in range(2):
                            mc = ntl[mt]
                            mo = noff[mt]
                            ps = psSB.tile([128, 2, NT], F32, tag="SB")
                            for hh in range(2):
                                base = hh * 64
                                nc.tensor.matmul(ps[:mc, hh, :], sgn_n[base:base + 64, 6 + hp_n, mo:mo + mc],
                                                 sgn_n[base:base + 64, hp_n, :], start=True, stop=True)
                                cidx = hh * 6 + hp_n
                                nc.scalar.activation(out=eA_n[mt][:mc, hh, :], in_=ps[:mc, hh, :],
                                                     func=mybir.ActivationFunctionType.Exp,
                                                     scale=cbc_n[:mc, cidx:cidx + 1])
                            nc.gpsimd.tensor_tensor(out=eA_n[mt][:mc], in0=eA_n[mt][:mc],
                                                    in1=ebT[mt][:mc, 2 * hp_n:2 * hp_n + 2, :],
                                                    op=mybir.AluOpType.mult)
                        eTs[s] = eA_n
                    if s >= LAG:
                        so = s - LAG
                        io, hpo = so % 2, so // 2
                        eA = eTs.pop(so)
                        rzv = rzs.pop(so)
                        vd = vd_i[io]
                        bc = psSB.tile([128, 2 * NT], F32, tag="SB")
                        nc.tensor.matmul(bc[:], onesr[:1], rzv[:1], start=True, stop=True)
                        pqT = [hpool.tile([128, 2, NT], BF16, tag=f"pqT{mt}", name=f"pqT{mt}_{so}")
                               for mt in range(2)]
                        for mt in range(2):
                            mc = ntl[mt]
                            bcv = bc[:mc].rearrange("p (h n) -> p h n", h=2)
                            nc.vector.tensor_tensor(out=eA[mt][:mc], in0=eA[mt][:mc],
                                                    in1=bcv, op=mybir.AluOpType.mult)
                            nc.vector.tensor_scalar(out=pqT[mt][:mc], in0=eA[mt][:mc],
                                                    scalar1=EXP2_23, scalar2=EXP2_23,
                                                    op0=mybir.AluOpType.add, op1=mybir.AluOpType.subtract)
                        ppv = psP.tile([128, NT], F32, tag="P")
                        for hh in range(2):
                            h = 2 * hpo + hh
                            base = hh * 64
                            for mt in range(2):
                                mc = ntl[mt]
                                nc.tensor.matmul(ppv[base:base + 64, :], vd[mt][:mc, h * 64:(h + 1) * 64],
                                                 pqT[mt][:mc, hh, :], start=(mt == 0), stop=(mt == 1))
                        nc.scalar.copy(attnT_i[io][:, hpo, :], ppv[:])
                        if so == 10:
                            emit_proj(0)
                        elif so == 11:
                            emit_proj(1)
    nc.compile()
    return nc


def _build_rel_index():
    H_IN = W_IN = 14
    coords = np.stack(np.meshgrid(np.arange(H_IN), np.arange(W_IN), indexing="ij"))
    flat = coords.reshape(2, -1)
    rel = flat[:, :, None] - flat[:, None, :]
    rel = rel.transpose(1, 2, 0).astype(np.int64)
    rel[:, :, 0] += H_IN - 1
    rel[:, :, 1] += W_IN - 1
    rel[:, :, 0] *= 2 * W_IN - 1
    idx = np.zeros((NT, NT), dtype=np.int64)
    idx[1:, 1:] = rel.sum(-1)
    idx[0, :] = NREL - 3
    idx[:, 0] = NREL - 2
    idx[0, 0] = NREL - 1
    return idx


def kernel(x, qkv_w, proj_w, proj_b, rel_bias_table, rel_index):
    x = np.asarray(x, dtype=np.float32)
    qkv_w = np.asarray(qkv_w, dtype=np.float32)
    proj_w = np.asarray(proj_w, dtype=np.float32)
    proj_b = np.asarray(proj_b, dtype=np.float32)
    rel_bias_table = np.asarray(rel_bias_table, dtype=np.float32)
    rel_index = np.asarray(rel_index)

    if "nc" not in _CACHE:
        _CACHE["nc"] = _build_nc()
    nc = _CACHE["nc"]

    W2 = np.ascontiguousarray(qkv_w.T)                      # (768, 2304)
    wh = W2.astype(bf)
    wl = (W2 - wh.astype(np.float32)).astype(bf)
    pw = np.ascontiguousarray(proj_w.T / 255.0).astype(bf)  # fold 1/255
    biasg = rel_bias_table[rel_index].transpose(2, 0, 1).astype(np.float32)  # (12,197,197) [h,n,m]
    ebias = np.ascontiguousarray(np.exp(biasg.transpose(0, 2, 1)))           # (12,197,197) [h,m,n]
    sel = np.zeros((128, 2), np.float32)
    sel[:64, 0] = 1.0
    sel[64:, 1] = 1.0

    in_maps = []
    for c in range(N_CORES):
        xc = x[c * BP:(c + 1) * BP].reshape(TOK, DIM)
        xT = np.ascontiguousarray(xc.T)                     # (768, 3152)
        xh = xT.astype(bf)
        xl = (xT - xh.astype(np.float32)).astype(bf)
        in_maps.append({
            "xh": xh, "xl": xl, "wh": wh, "wl": wl, "pw": pw,
            "pb": proj_b.astype(np.float32), "bias": ebias,
            "sel": sel, "onesc": np.ones((128, 1), np.float32),
            "onesr": np.ones((1, 128), np.float32),
            "onesb": np.ones((1, 128), dtype=bf),
            "pbb": proj_b.reshape(1, DIM).astype(bf),
        })

    global _LAST_IN_MAPS
    _LAST_IN_MAPS = in_maps
    res = run_bass_kernel_spmd(nc, in_maps, list(range(N_CORES)))
    out = np.concatenate(
        [res.results[c]["out"].reshape(BP, NT, DIM) for c in range(N_CORES)], axis=0)
    return out.astype(np.float32)
